# revision 2
# baseline (speedup 1.0000x reference)
"""Trainium2 Bass kernel for nn_AttnDecoder_87230785782556 — v3.

Multi-head attention decoder: out = softmax((xq Wq)(xk Wk)^T * s) (xv Wv) Wo
Sharding: 8 cores = 2 batches x 4 head-groups (tensor-parallel heads,
row-split Wo; partial outputs summed on host).

Pipeline design (per core):
- bf16 on-chip (PSUM f32): halves DMA + SBUF vs f32.
- scores [kv, q] in PSUM -> exp on Act engine (the rate limiter: ~33us
  per chunk) -> attnV with exp STATIONARY and vh moving (65 rows per
  matmul: half the PE rows of the classic orientation). The vh ones
  column emits softmax denominators into the same PSUM accumulator.
- normalize = per-partition scalar multiply on the Pool engine during
  PSUM evacuation; PE transpose returns attn to [feat, q] for the
  output projection.
- ALL non-loop PE work (K/V/Q projections, transposes, output
  projection) is sliced into small closures and interleaved into the
  Act-bound kv loops, including the NEXT repeat's projections, so the
  PE never idles at phase or repeat boundaries.
- output DMA rides the Pool DGE queue so a blocked x-load on the SP
  queue can never delay result writeback.
"""
import math
import numpy as np
import ml_dtypes

from concourse import bacc, mybir, tile
from concourse.bass_utils import run_bass_kernel_spmd

B = 2
SEQ = 2048
E = 1024
NUM_HEADS = 16
HD = 64
QK_SCALE = 0.125
N_CORES = 8
HPC = 4            # heads per core
P = 128
NQ = 512           # q chunk

F32 = mybir.dt.float32
BF16 = mybir.dt.bfloat16


def build_program(seq=SEQ, repeat=1):
    nc = bacc.Bacc("TRN2", target_bir_lowering=False, debug=False,
                   num_devices=N_CORES)

    n_qc = seq // NQ            # q chunks (4)
    n_kv = seq // P             # kv tiles of 128 (16)
    n_kt = E // P               # contraction tiles (8)
    FPC = HPC * HD              # features per core (256)
    n_m = FPC // P              # feature pair-tiles (2)
    NB = NQ // P                # q 128-blocks per chunk (4)
    VW = HPC * (HD + 1)         # vh row width (260)

    xtq = nc.dram_tensor("xtq", [E, seq], BF16, kind="ExternalInput")
    xtk = nc.dram_tensor("xtk", [E, seq], BF16, kind="ExternalInput")
    xtv = nc.dram_tensor("xtv", [E, seq], BF16, kind="ExternalInput")
    wq = nc.dram_tensor("wq", [E, FPC], BF16, kind="ExternalInput")
    wk = nc.dram_tensor("wk", [E, FPC], BF16, kind="ExternalInput")
    wv = nc.dram_tensor("wv", [E, VW], BF16, kind="ExternalInput")
    wo = nc.dram_tensor("wo", [FPC, E], BF16, kind="ExternalInput")
    ident = nc.dram_tensor("ident", [P, P], BF16, kind="ExternalInput")
    out = nc.dram_tensor("out", [seq, E], F32, kind="ExternalOutput")

    with tile.TileContext(nc) as tc, nc.allow_low_precision("bf16 pipeline"):
        import contextlib
        ctx = contextlib.ExitStack()
        with ctx:
            consts = ctx.enter_context(tc.tile_pool(name="consts", bufs=1))
            bigs = ctx.enter_context(tc.tile_pool(name="bigs", bufs=2))
            vhp = ctx.enter_context(tc.tile_pool(name="vhp", bufs=2 * n_kv))
            qhp = ctx.enter_context(tc.tile_pool(name="qhp", bufs=4))
            xs = ctx.enter_context(tc.tile_pool(name="xs", bufs=1))
            expp = ctx.enter_context(tc.tile_pool(name="expp", bufs=8))
            nrmp = ctx.enter_context(tc.tile_pool(name="nrmp", bufs=4))
            stkp = ctx.enter_context(tc.tile_pool(name="stkp", bufs=4))
            rcpp = ctx.enter_context(tc.tile_pool(name="rcpp", bufs=4))
            finp = ctx.enter_context(tc.tile_pool(name="finp", bufs=3))
            ps = ctx.enter_context(tc.tile_pool(name="ps", bufs=2, space="PSUM"))
            avp = ctx.enter_context(tc.tile_pool(name="avp", bufs=2, space="PSUM"))
            opp = ctx.enter_context(tc.tile_pool(name="opp", bufs=2, space="PSUM"))

            # ---- resident constants (wk first: K proj is the first consumer)
            wq_t = consts.tile([P, n_kt, FPC], BF16, name="wq_t", tag="wq")
            wk_t = consts.tile([P, n_kt, FPC], BF16, name="wk_t", tag="wk")
            wv_t = consts.tile([P, n_kt, VW], BF16, name="wv_t", tag="wv")
            wo_t = consts.tile([P, n_m, E], BF16, name="wo_t", tag="wo")
            id_t = consts.tile([P, P], BF16, name="id_t", tag="id")
            nc.sync.dma_start(out=wk_t, in_=wk.ap().rearrange("(t p) m -> p t m", p=P))

            deferred = []

            def pop_one():
                if deferred:
                    deferred.pop(0)()

            def flush():
                while deferred:
                    deferred.pop(0)()

            # ---------- closure factories (allocate tiles eagerly, emit
            # instructions when called) ----------
            def x_dma_closure(dram, tag):
                tiles = [xs.tile([P, seq], BF16, name=f"{tag}{kt}",
                                 tag=f"{tag}{kt}") for kt in range(n_kt)]

                def go():
                    for kt in range(n_kt):
                        nc.sync.dma_start(out=tiles[kt],
                                          in_=dram.ap()[P * kt:P * (kt + 1), :])
                return tiles, go

            def kproj_closures(xk_tiles):
                """khT[m] [128, seq] bf16; one closure per (nq, m)."""
                khT = [bigs.tile([P, seq], BF16, name=f"khT{m}", tag=f"khT{m}")
                       for m in range(n_m)]
                cls = []
                for nq in range(n_qc):
                    for m in range(n_m):
                        box = {}

                        def go_a(nq=nq, m=m, box=box):
                            box["pt"] = opp.tile([P, NQ], F32, name="op_t", tag="op")
                            for kt in range(n_kt // 2):
                                nc.tensor.matmul(
                                    box["pt"],
                                    wk_t[:, kt, P * m:P * (m + 1)],
                                    xk_tiles[kt][:, NQ * nq:NQ * (nq + 1)],
                                    start=(kt == 0), stop=False)

                        def go_b(nq=nq, m=m, box=box):
                            for kt in range(n_kt // 2, n_kt):
                                nc.tensor.matmul(
                                    box["pt"],
                                    wk_t[:, kt, P * m:P * (m + 1)],
                                    xk_tiles[kt][:, NQ * nq:NQ * (nq + 1)],
                                    start=False, stop=(kt == n_kt - 1))
                            nc.vector.tensor_copy(
                                khT[m][:, NQ * nq:NQ * (nq + 1)], box["pt"])
                        cls.append(go_a)
                        cls.append(go_b)
                return khT, cls

            def vproj_closures(xv_tiles):
                """vh tiles [128, 4, 65] bf16; one closure per kv tile."""
                vh_tiles = [vhp.tile([P, HPC, HD + 1], BF16, name=f"vh{i}",
                                     tag="vh") for i in range(n_kv)]
                cls = []
                for mk in range(n_kv):
                    def go(mk=mk):
                        pt = opp.tile([P, VW], F32, name="op_t", tag="op")
                        c0 = P * mk
                        for kt in range(n_kt):
                            nc.tensor.matmul(
                                pt,
                                xv_tiles[kt][:, c0:c0 + P],
                                wv_t[:, kt, :],
                                start=(kt == 0), stop=(kt == n_kt - 1))
                        nc.vector.tensor_copy(
                            vh_tiles[mk],
                            pt.rearrange("p (h c) -> p h c", h=HPC))
                        nc.gpsimd.tensor_scalar_add(
                            vh_tiles[mk][:, 0::2, HD],
                            vh_tiles[mk][:, 0::2, HD], 1.0)
                        nc.gpsimd.tensor_scalar_add(
                            vh_tiles[mk][:, 1::2, 0],
                            vh_tiles[mk][:, 1::2, 0], 1.0)
                    cls.append(go)
                return vh_tiles, cls

            def qproj_closures(xq_tiles, qc):
                qhT = [qhp.tile([P, NQ], BF16, name=f"qhT{m}", tag=f"qhT{m}")
                       for m in range(n_m)]
                cls = []
                for m in range(n_m):
                    box = {}

                    def go_a(m=m, box=box):
                        box["pt"] = opp.tile([P, NQ], F32, name="op_t", tag="op")
                        for kt in range(n_kt // 2):
                            nc.tensor.matmul(
                                box["pt"],
                                wq_t[:, kt, P * m:P * (m + 1)],
                                xq_tiles[kt][:, NQ * qc:NQ * (qc + 1)],
                                start=(kt == 0), stop=False)

                    def go_b(m=m, box=box):
                        for kt in range(n_kt // 2, n_kt):
                            nc.tensor.matmul(
                                box["pt"],
                                wq_t[:, kt, P * m:P * (m + 1)],
                                xq_tiles[kt][:, NQ * qc:NQ * (qc + 1)],
                                start=False, stop=(kt == n_kt - 1))
                        nc.vector.tensor_copy(qhT[m], box["pt"])
                    cls.append(go_a)
                    cls.append(go_b)
                return qhT, cls

            def transpose_closure(nrm_t, stk_t):
                def go():
                    tp_t = ps.tile([P, NB, P], BF16, name="tp_t", tag="ps")
                    for qb in range(NB):
                        # all 4 transposes share one PSUM bank: single
                        # accumulation group (start zeroes the zero region)
                        nc.tensor.matmul(
                            tp_t[:, qb, :], nrm_t[:, qb, :], id_t,
                            is_transpose=True,
                            start=(qb == 0), stop=(qb == NB - 1))
                    nc.vector.tensor_copy(
                        stk_t, tp_t.rearrange("p a b -> p (a b)"))
                return go

            def outproj_closures(stk_tiles, qc):
                cls = []
                for qs in range(NB):
                    box = {}

                    def go_a(qs=qs, box=box):
                        box["fin"] = finp.tile([P, 2, NQ], F32, name="fin_t",
                                               tag="fin")
                        op_ps = opp.tile([P, NQ], F32, name="op_t", tag="op")
                        for pair in range(n_m):
                            nc.tensor.matmul(
                                op_ps,
                                stk_tiles[pair][:, P * qs:P * (qs + 1)],
                                wo_t[:, pair, 0:NQ],
                                start=(pair == 0), stop=(pair == n_m - 1))
                        nc.vector.tensor_copy(box["fin"][:, 0, :], op_ps)

                    def go_b(qs=qs, box=box):
                        op_ps = opp.tile([P, NQ], F32, name="op_t", tag="op")
                        for pair in range(n_m):
                            nc.tensor.matmul(
                                op_ps,
                                stk_tiles[pair][:, P * qs:P * (qs + 1)],
                                wo_t[:, pair, NQ:2 * NQ],
                                start=(pair == 0), stop=(pair == n_m - 1))
                        nc.vector.tensor_copy(box["fin"][:, 1, :], op_ps)
                        r0 = NQ * qc + P * qs
                        nc.gpsimd.dma_start(
                            out=out.ap()[r0:r0 + P, :],
                            in_=box["fin"].rearrange("p a b -> p (a b)"))
                    cls.append(go_a)
                    cls.append(go_b)
                return cls

            def emit_pair_loop(khT, vh_tiles, qhT, pair):
                """scores -> exp -> attnV (lag 2) for one head pair; returns
                (nrm_t, stk_t) with normalize already emitted."""
                av = [avp.tile([P, 2, 2 * (HD + 1)], F32,
                               name=f"av{i}", tag="av") for i in range(2)]
                exp_tiles = []

                def emit_attnv(g, qbs=range(NB)):
                    # one accumulation group per av PSUM bank: start only on
                    # the tile's very first write (zeroes the whole 2KB zero
                    # region), stop only on its very last
                    for qb in qbs:
                        for h01 in range(2):
                            nc.tensor.matmul(
                                av[qb // 2][:, qb % 2,
                                            (HD + 1) * h01:(HD + 1) * (h01 + 1)],
                                exp_tiles[g][:, h01, P * qb:P * (qb + 1)],
                                vh_tiles[g][:, 2 * pair + h01, :],
                                start=(g == 0 and qb % 2 == 0 and h01 == 0),
                                stop=(g == n_kv - 1 and qb % 2 == 1 and h01 == 1))

                for g in range(n_kv):
                    sc_t = ps.tile([P, 2, NQ], F32, name="ps_t", tag="ps")
                    for h01 in range(2):
                        nc.tensor.matmul(
                            sc_t[:, h01, :],
                            khT[pair][HD * h01:HD * (h01 + 1), P * g:P * (g + 1)],
                            qhT[pair][HD * h01:HD * (h01 + 1), :],
                            start=True, stop=True,
                            tile_position=(HD * h01, 0))
                    e_t = expp.tile([P, 2, NQ], BF16, name="exp_t", tag="exp")
                    nc.scalar.activation(
                        e_t, sc_t, mybir.ActivationFunctionType.Exp)
                    exp_tiles.append(e_t)
                    if g >= 2:
                        emit_attnv(g - 2)
                    if g >= 3:
                        pop_one()

                rcp_t = rcpp.tile([P, 2, 2, 2], F32, name="rcp_t", tag="rcp")
                nrm_t = nrmp.tile([P, NB, P], BF16, name="nrm_t", tag="nrm")

                def emit_norm(i):
                    nc.vector.reciprocal(rcp_t[:, i, :, :], av[i][:, :, HD:HD + 2])
                    for qb in (2 * i, 2 * i + 1):
                        nc.vector.tensor_scalar_mul(
                            nrm_t[:, qb, 0:HD],
                            av[qb // 2][:, qb % 2, 0:HD],
                            rcp_t[:, qb // 2, qb % 2, 0:1])
                        nc.vector.tensor_scalar_mul(
                            nrm_t[:, qb, HD:2 * HD],
                            av[qb // 2][:, qb % 2, HD + 2:2 * (HD + 1)],
                            rcp_t[:, qb // 2, qb % 2, 1:2])

                emit_attnv(n_kv - 2)
                emit_attnv(n_kv - 1, qbs=(0, 1))
                emit_norm(0)
                emit_attnv(n_kv - 1, qbs=(2, 3))
                emit_norm(1)

                stk_t = stkp.tile([P, NQ], BF16, name="stk_t", tag="stk")
                return nrm_t, stk_t

            # ---------- program emission with cross-repeat pipelining ------
            xk_tiles, go_xk = x_dma_closure(xtk, "xk")
            go_xk()
            nc.sync.dma_start(out=wv_t, in_=wv.ap().rearrange("(t p) m -> p t m", p=P))
            xv_tiles, go_xv = x_dma_closure(xtv, "xv")
            go_xv()
            nc.sync.dma_start(out=wq_t, in_=wq.ap().rearrange("(t p) m -> p t m", p=P))
            xq_tiles, go_xq = x_dma_closure(xtq, "xq")
            go_xq()
            nc.sync.dma_start(out=wo_t, in_=wo.ap().rearrange("(t p) m -> p t m", p=P))
            nc.sync.dma_start(out=id_t, in_=ident.ap())

            khT, kcls = kproj_closures(xk_tiles)
            for c in kcls:
                c()
            vh_tiles, vcls = vproj_closures(xv_tiles)
            for c in vcls:
                c()

            qhT, qcls = qproj_closures(xq_tiles, 0)
            for c in qcls:
                c()

            for rep in range(repeat):
                for qc in range(n_qc):
                    if qc == 1 and rep + 1 < repeat:
                        # stage next repeat's loads + projections as deferred
                        nxk, go_nxk = x_dma_closure(xtk, "xk")
                        nxv, go_nxv = x_dma_closure(xtv, "xv")
                        nxq, go_nxq = x_dma_closure(xtq, "xq")
                        nkhT, nkcls = kproj_closures(nxk)
                        nvh, nvcls = vproj_closures(nxv)
                        deferred.append(go_nxk)
                        deferred.extend(nkcls)
                        deferred.append(go_nxv)
                        deferred.extend(nvcls)
                        deferred.append(go_nxq)
                        next_state = (nkhT, nvh, nxq)
                    stk_tiles = []
                    for pair in range(n_m):
                        if pair == 1:
                            # queue the NEXT chunk's Q projection so it pops
                            # inside this pair's loop (Act keeps streaming)
                            if qc + 1 < n_qc:
                                nqhT, nqcls = qproj_closures(xq_tiles, qc + 1)
                                deferred.extend(nqcls)
                            elif rep + 1 < repeat:
                                nqhT, nqcls = qproj_closures(next_state[2], 0)
                                deferred.extend(nqcls)
                            else:
                                nqhT = None
                        nrm_t, stk_t = emit_pair_loop(khT, vh_tiles, qhT, pair)
                        stk_tiles.append(stk_t)
                        deferred.append(transpose_closure(nrm_t, stk_t))
                    deferred.extend(outproj_closures(stk_tiles, qc))
                    qhT = nqhT
                # rep boundary: everything still pending must be emitted
                # before the next rep's chunks reference the new khT/vh
                flush()
                if rep + 1 < repeat:
                    khT, vh_tiles, xq_tiles = next_state
    nc.finalize()
    return nc


_PROG_CACHE = {}


def _get_program(seq=SEQ, repeat=1):
    key = (seq, repeat)
    if key not in _PROG_CACHE:
        _PROG_CACHE[key] = build_program(seq, repeat)
    return _PROG_CACHE[key]


def shard_inputs(q, k, v, Wq, Wk, Wv, Wo, seq=SEQ):
    """Build the 8 per-core input maps (host-side layout prep, bf16)."""
    bf = ml_dtypes.bfloat16
    scale = np.float32(QK_SCALE / math.sqrt(B))
    xt = {}
    for b in range(B):
        xt[b] = (np.ascontiguousarray(q[b][:seq].T).astype(bf),
                 np.ascontiguousarray(k[b][:seq].T).astype(bf),
                 np.ascontiguousarray(v[b][:seq].T).astype(bf))
    id_np = np.eye(P, dtype=np.float32).astype(bf)
    in_maps = []
    for c in range(N_CORES):
        b = c // 4
        hg = c % 4
        heads = [4 * hg + j for j in range(HPC)]
        wq_s = np.concatenate([Wq[:, h::NUM_HEADS] for h in heads], axis=1) * scale
        wk_s = np.concatenate([Wk[:, h::NUM_HEADS] for h in heads], axis=1)
        wv_s = np.zeros((E, HPC, HD + 1), dtype=np.float32)
        for j, h in enumerate(heads):
            if j % 2 == 0:
                wv_s[:, j, 0:HD] = Wv[:, h::NUM_HEADS]
            else:
                wv_s[:, j, 1:HD + 1] = Wv[:, h::NUM_HEADS]
        wo_s = np.concatenate([Wo[h::NUM_HEADS, :] for h in heads], axis=0)
        in_maps.append({
            "xtq": xt[b][0],
            "xtk": xt[b][1],
            "xtv": xt[b][2],
            "wq": np.ascontiguousarray(wq_s).astype(bf),
            "wk": np.ascontiguousarray(wk_s).astype(bf),
            "wv": np.ascontiguousarray(wv_s.reshape(E, HPC * (HD + 1))).astype(bf),
            "wo": np.ascontiguousarray(wo_s).astype(bf),
            "ident": id_np,
        })
    return in_maps


def unshard(results, seq=SEQ):
    out = np.zeros((B, seq, E), dtype=np.float32)
    for c in range(N_CORES):
        out[c // 4] += results[c]["out"]
    return out


def kernel(q, k, v, Wq, Wk, Wv, Wo):
    q = np.asarray(q, dtype=np.float32)
    k = np.asarray(k, dtype=np.float32)
    v = np.asarray(v, dtype=np.float32)
    Wq = np.asarray(Wq, dtype=np.float32)
    Wk = np.asarray(Wk, dtype=np.float32)
    Wv = np.asarray(Wv, dtype=np.float32)
    Wo = np.asarray(Wo, dtype=np.float32)
    nc = _get_program()
    in_maps = shard_inputs(q, k, v, Wq, Wk, Wv, Wo)
    res = run_bass_kernel_spmd(nc, in_maps, list(range(N_CORES)))
    return unshard(res.results)


# revision 3
# speedup vs baseline: 1.0508x; 1.0508x over previous
"""Trainium2 Bass kernel for nn_AttnDecoder_87230785782556 — v3.

Multi-head attention decoder: out = softmax((xq Wq)(xk Wk)^T * s) (xv Wv) Wo
Sharding: 8 cores = 2 batches x 4 head-groups (tensor-parallel heads,
row-split Wo; partial outputs summed on host).

Pipeline design (per core):
- bf16 on-chip (PSUM f32): halves DMA + SBUF vs f32.
- scores [kv, q] in PSUM -> exp on Act engine (the rate limiter: ~33us
  per chunk) -> attnV with exp STATIONARY and vh moving (65 rows per
  matmul: half the PE rows of the classic orientation). The vh ones
  column emits softmax denominators into the same PSUM accumulator.
- normalize = per-partition scalar multiply on the Pool engine during
  PSUM evacuation; PE transpose returns attn to [feat, q] for the
  output projection.
- ALL non-loop PE work (K/V/Q projections, transposes, output
  projection) is sliced into small closures and interleaved into the
  Act-bound kv loops, including the NEXT repeat's projections, so the
  PE never idles at phase or repeat boundaries.
- output DMA rides the Pool DGE queue so a blocked x-load on the SP
  queue can never delay result writeback.
"""
import math
import numpy as np
import ml_dtypes

from concourse import bacc, mybir, tile
from concourse.bass_utils import run_bass_kernel_spmd

B = 2
SEQ = 2048
E = 1024
NUM_HEADS = 16
HD = 64
QK_SCALE = 0.125
N_CORES = 8
HPC = 4            # heads per core
P = 128
NQ = 512           # q chunk

F32 = mybir.dt.float32
BF16 = mybir.dt.bfloat16


def build_program(seq=SEQ, repeat=1):
    nc = bacc.Bacc("TRN2", target_bir_lowering=False, debug=False,
                   num_devices=N_CORES)

    n_qc = seq // NQ            # q chunks (4)
    n_kv = seq // P             # kv tiles of 128 (16)
    n_kt = E // P               # contraction tiles (8)
    FPC = HPC * HD              # features per core (256)
    n_m = FPC // P              # feature pair-tiles (2)
    NB = NQ // P                # q 128-blocks per chunk (4)
    VW = HPC * (HD + 1)         # vh row width (260)

    xtq = nc.dram_tensor("xtq", [E, seq], BF16, kind="ExternalInput")
    xtk = nc.dram_tensor("xtk", [E, seq], BF16, kind="ExternalInput")
    xtv = nc.dram_tensor("xtv", [E, seq], BF16, kind="ExternalInput")
    wq = nc.dram_tensor("wq", [E, FPC], BF16, kind="ExternalInput")
    wk = nc.dram_tensor("wk", [E, FPC], BF16, kind="ExternalInput")
    wv = nc.dram_tensor("wv", [E, VW], BF16, kind="ExternalInput")
    wo = nc.dram_tensor("wo", [FPC, E], BF16, kind="ExternalInput")
    ident = nc.dram_tensor("ident", [P, P], BF16, kind="ExternalInput")
    out = nc.dram_tensor("out", [seq, E], F32, kind="ExternalOutput")

    with tile.TileContext(nc) as tc, nc.allow_low_precision("bf16 pipeline"):
        import contextlib
        ctx = contextlib.ExitStack()
        with ctx:
            consts = ctx.enter_context(tc.tile_pool(name="consts", bufs=1))
            bigs = ctx.enter_context(tc.tile_pool(name="bigs", bufs=2))
            vhp = ctx.enter_context(tc.tile_pool(name="vhp", bufs=2 * n_kv))
            qhp = ctx.enter_context(tc.tile_pool(name="qhp", bufs=4))
            xs = ctx.enter_context(tc.tile_pool(name="xs", bufs=1))
            expp = ctx.enter_context(tc.tile_pool(name="expp", bufs=8))
            nrmp = ctx.enter_context(tc.tile_pool(name="nrmp", bufs=4))
            stkp = ctx.enter_context(tc.tile_pool(name="stkp", bufs=4))
            rcpp = ctx.enter_context(tc.tile_pool(name="rcpp", bufs=4))
            finp = ctx.enter_context(tc.tile_pool(name="finp", bufs=3))
            ps = ctx.enter_context(tc.tile_pool(name="ps", bufs=2, space="PSUM"))
            avp = ctx.enter_context(tc.tile_pool(name="avp", bufs=2, space="PSUM"))
            opp = ctx.enter_context(tc.tile_pool(name="opp", bufs=2, space="PSUM"))

            # ---- resident constants (wk first: K proj is the first consumer)
            wq_t = consts.tile([P, n_kt, FPC], BF16, name="wq_t", tag="wq")
            wk_t = consts.tile([P, n_kt, FPC], BF16, name="wk_t", tag="wk")
            wv_t = consts.tile([P, n_kt, VW], BF16, name="wv_t", tag="wv")
            wo_t = consts.tile([P, n_m, E], BF16, name="wo_t", tag="wo")
            id_t = consts.tile([P, P], BF16, name="id_t", tag="id")
            nc.sync.dma_start(out=wk_t, in_=wk.ap().rearrange("(t p) m -> p t m", p=P))

            deferred = []
            prereqs_done = [False]

            def pop_one():
                if deferred:
                    deferred.pop(0)()

            def flush():
                while deferred:
                    deferred.pop(0)()

            # ---------- closure factories (allocate tiles eagerly, emit
            # instructions when called) ----------
            def x_dma_closure(dram, tag):
                tiles = [xs.tile([P, seq], BF16, name=f"{tag}{kt}",
                                 tag=f"{tag}{kt}") for kt in range(n_kt)]

                def go():
                    for kt in range(n_kt):
                        nc.sync.dma_start(out=tiles[kt],
                                          in_=dram.ap()[P * kt:P * (kt + 1), :])
                return tiles, go

            def kproj_closures(xk_tiles):
                """khT[m] [128, seq] bf16; one closure per (nq, m)."""
                khT = [bigs.tile([P, seq], BF16, name=f"khT{m}", tag=f"khT{m}")
                       for m in range(n_m)]
                cls = []
                for nq in range(n_qc):
                    for m in range(n_m):
                        box = {}

                        def go_a(nq=nq, m=m, box=box):
                            box["pt"] = opp.tile([P, NQ], F32, name="op_t", tag="op")
                            for kt in range(n_kt // 2):
                                nc.tensor.matmul(
                                    box["pt"],
                                    wk_t[:, kt, P * m:P * (m + 1)],
                                    xk_tiles[kt][:, NQ * nq:NQ * (nq + 1)],
                                    start=(kt == 0), stop=False)

                        def go_b(nq=nq, m=m, box=box):
                            for kt in range(n_kt // 2, n_kt):
                                nc.tensor.matmul(
                                    box["pt"],
                                    wk_t[:, kt, P * m:P * (m + 1)],
                                    xk_tiles[kt][:, NQ * nq:NQ * (nq + 1)],
                                    start=False, stop=(kt == n_kt - 1))
                            nc.vector.tensor_copy(
                                khT[m][:, NQ * nq:NQ * (nq + 1)], box["pt"])
                        cls.append(go_a)
                        cls.append(go_b)
                return khT, cls

            def vproj_closures(xv_tiles):
                """vh tiles [128, 4, 65] bf16; one closure per kv tile."""
                vh_tiles = [vhp.tile([P, HPC, HD + 1], BF16, name=f"vh{i}",
                                     tag="vh") for i in range(n_kv)]
                cls = []
                for mk in range(n_kv):
                    def go(mk=mk):
                        pt = opp.tile([P, VW], F32, name="op_t", tag="op")
                        c0 = P * mk
                        for kt in range(n_kt):
                            nc.tensor.matmul(
                                pt,
                                xv_tiles[kt][:, c0:c0 + P],
                                wv_t[:, kt, :],
                                start=(kt == 0), stop=(kt == n_kt - 1))
                        nc.vector.tensor_copy(
                            vh_tiles[mk],
                            pt.rearrange("p (h c) -> p h c", h=HPC))
                        nc.gpsimd.tensor_scalar_add(
                            vh_tiles[mk][:, 0::2, HD],
                            vh_tiles[mk][:, 0::2, HD], 1.0)
                        nc.gpsimd.tensor_scalar_add(
                            vh_tiles[mk][:, 1::2, 0],
                            vh_tiles[mk][:, 1::2, 0], 1.0)
                    cls.append(go)
                return vh_tiles, cls

            def qproj_closures(xq_tiles, qc):
                qhT = [qhp.tile([P, NQ], BF16, name=f"qhT{m}", tag=f"qhT{m}")
                       for m in range(n_m)]
                cls = []
                for m in range(n_m):
                    box = {}

                    def go_a(m=m, box=box):
                        box["pt"] = opp.tile([P, NQ], F32, name="op_t", tag="op")
                        for kt in range(n_kt // 2):
                            nc.tensor.matmul(
                                box["pt"],
                                wq_t[:, kt, P * m:P * (m + 1)],
                                xq_tiles[kt][:, NQ * qc:NQ * (qc + 1)],
                                start=(kt == 0), stop=False)

                    def go_b(m=m, box=box):
                        for kt in range(n_kt // 2, n_kt):
                            nc.tensor.matmul(
                                box["pt"],
                                wq_t[:, kt, P * m:P * (m + 1)],
                                xq_tiles[kt][:, NQ * qc:NQ * (qc + 1)],
                                start=False, stop=(kt == n_kt - 1))
                        nc.vector.tensor_copy(qhT[m], box["pt"])
                    cls.append(go_a)
                    cls.append(go_b)
                return qhT, cls

            def transpose_closure(nrm_t, stk_t):
                def go():
                    tp_t = ps.tile([P, NB, P], BF16, name="tp_t", tag="ps")
                    for qb in range(NB):
                        # all 4 transposes share one PSUM bank: single
                        # accumulation group (start zeroes the zero region)
                        nc.tensor.matmul(
                            tp_t[:, qb, :], nrm_t[:, qb, :], id_t,
                            is_transpose=True,
                            start=(qb == 0), stop=(qb == NB - 1))
                    nc.vector.tensor_copy(
                        stk_t, tp_t.rearrange("p a b -> p (a b)"))
                return go

            def outproj_closures(stk_tiles, qc):
                cls = []
                for qs in range(NB):
                    box = {}

                    def go_a(qs=qs, box=box):
                        box["fin"] = finp.tile([P, 2, NQ], F32, name="fin_t",
                                               tag="fin")
                        op_ps = opp.tile([P, NQ], F32, name="op_t", tag="op")
                        for pair in range(n_m):
                            nc.tensor.matmul(
                                op_ps,
                                stk_tiles[pair][:, P * qs:P * (qs + 1)],
                                wo_t[:, pair, 0:NQ],
                                start=(pair == 0), stop=(pair == n_m - 1))
                        nc.vector.tensor_copy(box["fin"][:, 0, :], op_ps)

                    def go_b(qs=qs, box=box):
                        op_ps = opp.tile([P, NQ], F32, name="op_t", tag="op")
                        for pair in range(n_m):
                            nc.tensor.matmul(
                                op_ps,
                                stk_tiles[pair][:, P * qs:P * (qs + 1)],
                                wo_t[:, pair, NQ:2 * NQ],
                                start=(pair == 0), stop=(pair == n_m - 1))
                        nc.vector.tensor_copy(box["fin"][:, 1, :], op_ps)
                        r0 = NQ * qc + P * qs
                        nc.gpsimd.dma_start(
                            out=out.ap()[r0:r0 + P, :],
                            in_=box["fin"].rearrange("p a b -> p (a b)"))
                    cls.append(go_a)
                    cls.append(go_b)
                return cls

            def emit_pair_loop(khT, vh_tiles, qhT, pair):
                """scores -> exp -> attnV (lag 2) for one head pair; returns
                (nrm_t, stk_t) with normalize already emitted."""
                av = [avp.tile([P, 2, 2 * (HD + 1)], F32,
                               name=f"av{i}", tag="av") for i in range(2)]
                exp_tiles = []

                def emit_attnv(g, qbs=range(NB)):
                    # one accumulation group per av PSUM bank: start only on
                    # the tile's very first write (zeroes the whole 2KB zero
                    # region), stop only on its very last
                    for qb in qbs:
                        for h01 in range(2):
                            nc.tensor.matmul(
                                av[qb // 2][:, qb % 2,
                                            (HD + 1) * h01:(HD + 1) * (h01 + 1)],
                                exp_tiles[g][:, h01, P * qb:P * (qb + 1)],
                                vh_tiles[g][:, 2 * pair + h01, :],
                                start=(g == 0 and qb % 2 == 0 and h01 == 0),
                                stop=(g == n_kv - 1 and qb % 2 == 1 and h01 == 1))

                for g in range(n_kv):
                    sc_t = ps.tile([P, 2, NQ], F32, name="ps_t", tag="ps")
                    for h01 in range(2):
                        nc.tensor.matmul(
                            sc_t[:, h01, :],
                            khT[pair][HD * h01:HD * (h01 + 1), P * g:P * (g + 1)],
                            qhT[pair][HD * h01:HD * (h01 + 1), :],
                            start=True, stop=True,
                            tile_position=(HD * h01, 0))
                    e_t = expp.tile([P, 2, NQ], BF16, name="exp_t", tag="exp")
                    nc.scalar.activation(
                        e_t, sc_t, mybir.ActivationFunctionType.Exp)
                    exp_tiles.append(e_t)
                    if g >= 2:
                        emit_attnv(g - 2)
                    if g >= 3:
                        pop_one()

                rcp_t = rcpp.tile([P, 2, 2, 2], F32, name="rcp_t", tag="rcp")
                nrm_t = nrmp.tile([P, NB, P], BF16, name="nrm_t", tag="nrm")

                def emit_norm(i):
                    nc.vector.reciprocal(rcp_t[:, i, :, :], av[i][:, :, HD:HD + 2])
                    for qb in (2 * i, 2 * i + 1):
                        nc.vector.tensor_scalar_mul(
                            nrm_t[:, qb, 0:HD],
                            av[qb // 2][:, qb % 2, 0:HD],
                            rcp_t[:, qb // 2, qb % 2, 0:1])
                        nc.vector.tensor_scalar_mul(
                            nrm_t[:, qb, HD:2 * HD],
                            av[qb // 2][:, qb % 2, HD + 2:2 * (HD + 1)],
                            rcp_t[:, qb // 2, qb % 2, 1:2])

                emit_attnv(n_kv - 2)
                emit_attnv(n_kv - 1, qbs=(0, 1))
                emit_norm(0)
                emit_attnv(n_kv - 1, qbs=(2, 3))
                emit_norm(1)

                stk_t = stkp.tile([P, NQ], BF16, name="stk_t", tag="stk")
                return nrm_t, stk_t

            # ---------- program emission with cross-repeat pipelining ------
            xk_tiles, go_xk = x_dma_closure(xtk, "xk")
            go_xk()
            nc.sync.dma_start(out=wv_t, in_=wv.ap().rearrange("(t p) m -> p t m", p=P))
            xv_tiles, go_xv = x_dma_closure(xtv, "xv")
            go_xv()
            nc.sync.dma_start(out=wq_t, in_=wq.ap().rearrange("(t p) m -> p t m", p=P))
            xq_tiles, go_xq = x_dma_closure(xtq, "xq")
            go_xq()
            nc.sync.dma_start(out=wo_t, in_=wo.ap().rearrange("(t p) m -> p t m", p=P))
            nc.sync.dma_start(out=id_t, in_=ident.ap())

            khT, kcls = kproj_closures(xk_tiles)
            for c in kcls:
                c()
            vh_tiles, vcls = vproj_closures(xv_tiles)
            for c in vcls:
                c()

            qhT, qcls = qproj_closures(xq_tiles, 0)
            for c in qcls:
                c()

            for rep in range(repeat):
                for qc in range(n_qc):
                    if qc == 1 and rep + 1 < repeat:
                        # stage next repeat's loads + projections as deferred
                        nxk, go_nxk = x_dma_closure(xtk, "xk")
                        nxv, go_nxv = x_dma_closure(xtv, "xv")
                        nxq, go_nxq = x_dma_closure(xtq, "xq")
                        nkhT, nkcls = kproj_closures(nxk)
                        nvh, nvcls = vproj_closures(nxv)
                        deferred.append(go_nxk)
                        deferred.extend(nkcls)
                        deferred.append(go_nxv)
                        deferred.extend(nvcls)
                        deferred.append(go_nxq)
                        next_state = (nkhT, nvh, nxq)
                    stk_tiles = []
                    for pair in range(n_m):
                        if pair == 1:
                            # queue the NEXT chunk's Q projection so it pops
                            # inside this pair's loop (Act keeps streaming)
                            if qc + 1 < n_qc:
                                nqhT, nqcls = qproj_closures(xq_tiles, qc + 1)
                                deferred.extend(nqcls)
                            elif rep + 1 < repeat:
                                nqhT, nqcls = qproj_closures(next_state[2], 0)
                                deferred.extend(nqcls)
                                deferred.append(
                                    lambda: prereqs_done.__setitem__(0, True))
                            else:
                                nqhT = None
                        nrm_t, stk_t = emit_pair_loop(khT, vh_tiles, qhT, pair)
                        stk_tiles.append(stk_t)
                        deferred.append(transpose_closure(nrm_t, stk_t))
                    deferred.extend(outproj_closures(stk_tiles, qc))
                    qhT = nqhT
                # rep boundary: drain only what the next rep's first loop
                # depends on (khT/vh/x DMAs/qproj are FIFO-ahead of the
                # barrier); the remaining tail work pops inside the next
                # rep's chunk-0 loops, keeping the PE fed there.
                if rep + 1 < repeat:
                    while not prereqs_done[0]:
                        pop_one()
                    prereqs_done[0] = False
                    khT, vh_tiles, xq_tiles = next_state
                else:
                    flush()
    nc.finalize()
    return nc


_PROG_CACHE = {}


def _get_program(seq=SEQ, repeat=1):
    key = (seq, repeat)
    if key not in _PROG_CACHE:
        _PROG_CACHE[key] = build_program(seq, repeat)
    return _PROG_CACHE[key]


def shard_inputs(q, k, v, Wq, Wk, Wv, Wo, seq=SEQ):
    """Build the 8 per-core input maps (host-side layout prep, bf16)."""
    bf = ml_dtypes.bfloat16
    scale = np.float32(QK_SCALE / math.sqrt(B))
    xt = {}
    for b in range(B):
        xt[b] = (np.ascontiguousarray(q[b][:seq].T).astype(bf),
                 np.ascontiguousarray(k[b][:seq].T).astype(bf),
                 np.ascontiguousarray(v[b][:seq].T).astype(bf))
    id_np = np.eye(P, dtype=np.float32).astype(bf)
    in_maps = []
    for c in range(N_CORES):
        b = c // 4
        hg = c % 4
        heads = [4 * hg + j for j in range(HPC)]
        wq_s = np.concatenate([Wq[:, h::NUM_HEADS] for h in heads], axis=1) * scale
        wk_s = np.concatenate([Wk[:, h::NUM_HEADS] for h in heads], axis=1)
        wv_s = np.zeros((E, HPC, HD + 1), dtype=np.float32)
        for j, h in enumerate(heads):
            if j % 2 == 0:
                wv_s[:, j, 0:HD] = Wv[:, h::NUM_HEADS]
            else:
                wv_s[:, j, 1:HD + 1] = Wv[:, h::NUM_HEADS]
        wo_s = np.concatenate([Wo[h::NUM_HEADS, :] for h in heads], axis=0)
        in_maps.append({
            "xtq": xt[b][0],
            "xtk": xt[b][1],
            "xtv": xt[b][2],
            "wq": np.ascontiguousarray(wq_s).astype(bf),
            "wk": np.ascontiguousarray(wk_s).astype(bf),
            "wv": np.ascontiguousarray(wv_s.reshape(E, HPC * (HD + 1))).astype(bf),
            "wo": np.ascontiguousarray(wo_s).astype(bf),
            "ident": id_np,
        })
    return in_maps


def unshard(results, seq=SEQ):
    out = np.zeros((B, seq, E), dtype=np.float32)
    for c in range(N_CORES):
        out[c // 4] += results[c]["out"]
    return out


def kernel(q, k, v, Wq, Wk, Wv, Wo):
    q = np.asarray(q, dtype=np.float32)
    k = np.asarray(k, dtype=np.float32)
    v = np.asarray(v, dtype=np.float32)
    Wq = np.asarray(Wq, dtype=np.float32)
    Wk = np.asarray(Wk, dtype=np.float32)
    Wv = np.asarray(Wv, dtype=np.float32)
    Wo = np.asarray(Wo, dtype=np.float32)
    nc = _get_program()
    in_maps = shard_inputs(q, k, v, Wq, Wk, Wv, Wo)
    res = run_bass_kernel_spmd(nc, in_maps, list(range(N_CORES)))
    return unshard(res.results)


# revision 4
# speedup vs baseline: 1.1546x; 1.0988x over previous
"""Trainium2 Bass kernel for nn_AttnDecoder_87230785782556 — v3.

Multi-head attention decoder: out = softmax((xq Wq)(xk Wk)^T * s) (xv Wv) Wo
Sharding: 8 cores = 2 batches x 4 head-groups (tensor-parallel heads,
row-split Wo; partial outputs summed on host).

Pipeline design (per core):
- bf16 on-chip (PSUM f32): halves DMA + SBUF vs f32.
- scores [kv, q] in PSUM -> exp on Act engine (the rate limiter: ~33us
  per chunk) -> attnV with exp STATIONARY and vh moving (65 rows per
  matmul: half the PE rows of the classic orientation). The vh ones
  column emits softmax denominators into the same PSUM accumulator.
- normalize = per-partition scalar multiply on the Pool engine during
  PSUM evacuation; PE transpose returns attn to [feat, q] for the
  output projection.
- ALL non-loop PE work (K/V/Q projections, transposes, output
  projection) is sliced into small closures and interleaved into the
  Act-bound kv loops, including the NEXT repeat's projections, so the
  PE never idles at phase or repeat boundaries.
- output DMA rides the Pool DGE queue so a blocked x-load on the SP
  queue can never delay result writeback.
"""
import math
import numpy as np
import ml_dtypes

from concourse import bacc, mybir, tile
from concourse.bass_utils import run_bass_kernel_spmd

B = 2
SEQ = 2048
E = 1024
NUM_HEADS = 16
HD = 64
QK_SCALE = 0.125
N_CORES = 8
HPC = 4            # heads per core
P = 128
NQ = 512           # q chunk

F32 = mybir.dt.float32
BF16 = mybir.dt.bfloat16


def build_program(seq=SEQ, repeat=1):
    nc = bacc.Bacc("TRN2", target_bir_lowering=False, debug=False,
                   num_devices=N_CORES)

    n_qc = seq // NQ            # q chunks (4)
    n_kv = seq // P             # kv tiles of 128 (16)
    n_kt = E // P               # contraction tiles (8)
    FPC = HPC * HD              # features per core (256)
    n_m = FPC // P              # feature pair-tiles (2)
    NB = NQ // P                # q 128-blocks per chunk (4)
    VW = HPC * (HD + 1)         # vh row width (260)

    xtq = nc.dram_tensor("xtq", [E, seq], BF16, kind="ExternalInput")
    xtk = nc.dram_tensor("xtk", [E, seq], BF16, kind="ExternalInput")
    xtv = nc.dram_tensor("xtv", [E, seq], BF16, kind="ExternalInput")
    wq = nc.dram_tensor("wq", [E, FPC], BF16, kind="ExternalInput")
    wk = nc.dram_tensor("wk", [E, FPC], BF16, kind="ExternalInput")
    wv = nc.dram_tensor("wv", [E, VW], BF16, kind="ExternalInput")
    wo = nc.dram_tensor("wo", [FPC, E], BF16, kind="ExternalInput")
    ident = nc.dram_tensor("ident", [P, P], BF16, kind="ExternalInput")
    out = nc.dram_tensor("out", [seq, E], F32, kind="ExternalOutput")

    with tile.TileContext(nc) as tc, nc.allow_low_precision("bf16 pipeline"):
        import contextlib
        ctx = contextlib.ExitStack()
        with ctx:
            consts = ctx.enter_context(tc.tile_pool(name="consts", bufs=1))
            bigs = ctx.enter_context(tc.tile_pool(name="bigs", bufs=2))
            vhp = ctx.enter_context(tc.tile_pool(name="vhp", bufs=2 * n_kv))
            qhp = ctx.enter_context(tc.tile_pool(name="qhp", bufs=4))
            xs = ctx.enter_context(tc.tile_pool(name="xs", bufs=1))
            expp = ctx.enter_context(tc.tile_pool(name="expp", bufs=8))
            nrmp = ctx.enter_context(tc.tile_pool(name="nrmp", bufs=4))
            stkp = ctx.enter_context(tc.tile_pool(name="stkp", bufs=4))
            rcpp = ctx.enter_context(tc.tile_pool(name="rcpp", bufs=4))
            finp = ctx.enter_context(tc.tile_pool(name="finp", bufs=3))
            ps = ctx.enter_context(tc.tile_pool(name="ps", bufs=2, space="PSUM"))
            avp = ctx.enter_context(tc.tile_pool(name="avp", bufs=2, space="PSUM"))
            opp = ctx.enter_context(tc.tile_pool(name="opp", bufs=2, space="PSUM"))

            # ---- resident constants (wk first: K proj is the first consumer)
            wq_t = consts.tile([P, n_kt, FPC], BF16, name="wq_t", tag="wq")
            wk_t = consts.tile([P, n_kt, FPC], BF16, name="wk_t", tag="wk")
            wv_t = consts.tile([P, n_kt, VW], BF16, name="wv_t", tag="wv")
            wo_t = consts.tile([P, n_m, E], BF16, name="wo_t", tag="wo")
            id_t = consts.tile([P, P], BF16, name="id_t", tag="id")
            nc.sync.dma_start(out=wk_t, in_=wk.ap().rearrange("(t p) m -> p t m", p=P))

            deferred = []
            prereqs_done = [False]

            def pop_one():
                if deferred:
                    deferred.pop(0)()

            def flush():
                while deferred:
                    deferred.pop(0)()

            # ---------- closure factories (allocate tiles eagerly, emit
            # instructions when called) ----------
            def x_dma_closure(dram, tag):
                tiles = [xs.tile([P, seq], BF16, name=f"{tag}{kt}",
                                 tag=f"{tag}{kt}") for kt in range(n_kt)]

                def go():
                    for kt in range(n_kt):
                        nc.sync.dma_start(out=tiles[kt],
                                          in_=dram.ap()[P * kt:P * (kt + 1), :])
                return tiles, go

            def kproj_closures(xk_tiles):
                """khT[m] [128, seq] bf16; one closure per (nq, m)."""
                khT = [bigs.tile([P, seq], BF16, name=f"khT{m}", tag=f"khT{m}")
                       for m in range(n_m)]
                cls = []
                for nq in range(n_qc):
                    for m in range(n_m):
                        box = {}

                        def go_a(nq=nq, m=m, box=box):
                            box["pt"] = opp.tile([P, NQ], F32, name="op_t", tag="op")
                            for kt in range(n_kt // 2):
                                nc.tensor.matmul(
                                    box["pt"],
                                    wk_t[:, kt, P * m:P * (m + 1)],
                                    xk_tiles[kt][:, NQ * nq:NQ * (nq + 1)],
                                    start=(kt == 0), stop=False)

                        def go_b(nq=nq, m=m, box=box):
                            for kt in range(n_kt // 2, n_kt):
                                nc.tensor.matmul(
                                    box["pt"],
                                    wk_t[:, kt, P * m:P * (m + 1)],
                                    xk_tiles[kt][:, NQ * nq:NQ * (nq + 1)],
                                    start=False, stop=(kt == n_kt - 1))
                            nc.vector.tensor_copy(
                                khT[m][:, NQ * nq:NQ * (nq + 1)], box["pt"])
                        cls.append(go_a)
                        cls.append(go_b)
                return khT, cls

            def vproj_closures(xv_tiles):
                """vh tiles [128, 4, 65] bf16; one closure per kv tile."""
                vh_tiles = [vhp.tile([P, HPC, HD + 1], BF16, name=f"vh{i}",
                                     tag="vh") for i in range(n_kv)]
                cls = []
                for mk in range(n_kv):
                    def go(mk=mk):
                        pt = opp.tile([P, VW], F32, name="op_t", tag="op")
                        c0 = P * mk
                        for kt in range(n_kt):
                            nc.tensor.matmul(
                                pt,
                                xv_tiles[kt][:, c0:c0 + P],
                                wv_t[:, kt, :],
                                start=(kt == 0), stop=(kt == n_kt - 1))
                        nc.vector.tensor_copy(
                            vh_tiles[mk],
                            pt.rearrange("p (h c) -> p h c", h=HPC))
                        nc.gpsimd.tensor_scalar_add(
                            vh_tiles[mk][:, 0::2, HD],
                            vh_tiles[mk][:, 0::2, HD], 1.0)
                        nc.gpsimd.tensor_scalar_add(
                            vh_tiles[mk][:, 1::2, 0],
                            vh_tiles[mk][:, 1::2, 0], 1.0)
                    cls.append(go)
                return vh_tiles, cls

            def qproj_closures(xq_tiles, qc):
                qhT = [qhp.tile([P, NQ], BF16, name=f"qhT{m}", tag=f"qhT{m}")
                       for m in range(n_m)]
                cls = []
                for m in range(n_m):
                    box = {}

                    def go_a(m=m, box=box):
                        box["pt"] = opp.tile([P, NQ], F32, name="op_t", tag="op")
                        for kt in range(n_kt // 2):
                            nc.tensor.matmul(
                                box["pt"],
                                wq_t[:, kt, P * m:P * (m + 1)],
                                xq_tiles[kt][:, NQ * qc:NQ * (qc + 1)],
                                start=(kt == 0), stop=False)

                    def go_b(m=m, box=box):
                        for kt in range(n_kt // 2, n_kt):
                            nc.tensor.matmul(
                                box["pt"],
                                wq_t[:, kt, P * m:P * (m + 1)],
                                xq_tiles[kt][:, NQ * qc:NQ * (qc + 1)],
                                start=False, stop=(kt == n_kt - 1))
                        nc.vector.tensor_copy(qhT[m], box["pt"])
                    cls.append(go_a)
                    cls.append(go_b)
                return qhT, cls

            def transpose_closure(nrm_t, stk_t):
                def go():
                    tp_t = opp.tile([P, NB, P], BF16, name="tp_t", tag="op")
                    for qb in range(NB):
                        # all 4 transposes share one PSUM bank: single
                        # accumulation group (start zeroes the zero region)
                        nc.tensor.matmul(
                            tp_t[:, qb, :], nrm_t[:, qb, :], id_t,
                            is_transpose=True,
                            start=(qb == 0), stop=(qb == NB - 1))
                    nc.vector.tensor_copy(
                        stk_t, tp_t.rearrange("p a b -> p (a b)"))
                return go

            def outproj_closures(stk_tiles, qc):
                cls = []
                for qs in range(NB):
                    box = {}

                    def go_a(qs=qs, box=box):
                        box["fin"] = finp.tile([P, 2, NQ], F32, name="fin_t",
                                               tag="fin")
                        op_ps = opp.tile([P, NQ], F32, name="op_t", tag="op")
                        for pair in range(n_m):
                            nc.tensor.matmul(
                                op_ps,
                                stk_tiles[pair][:, P * qs:P * (qs + 1)],
                                wo_t[:, pair, 0:NQ],
                                start=(pair == 0), stop=(pair == n_m - 1))
                        nc.vector.tensor_copy(box["fin"][:, 0, :], op_ps)

                    def go_b(qs=qs, box=box):
                        op_ps = opp.tile([P, NQ], F32, name="op_t", tag="op")
                        for pair in range(n_m):
                            nc.tensor.matmul(
                                op_ps,
                                stk_tiles[pair][:, P * qs:P * (qs + 1)],
                                wo_t[:, pair, NQ:2 * NQ],
                                start=(pair == 0), stop=(pair == n_m - 1))
                        nc.vector.tensor_copy(box["fin"][:, 1, :], op_ps)
                        r0 = NQ * qc + P * qs
                        nc.gpsimd.dma_start(
                            out=out.ap()[r0:r0 + P, :],
                            in_=box["fin"].rearrange("p a b -> p (a b)"))
                    cls.append(go_a)
                    cls.append(go_b)
                return cls

            def emit_pair_loop(khT, vh_tiles, qhT, pair):
                """scores -> exp -> attnV (lag 2) for one head pair; returns
                (nrm_t, stk_t) with normalize already emitted."""
                av = [avp.tile([P, 2, 2 * (HD + 1)], F32,
                               name=f"av{i}", tag="av") for i in range(2)]
                exp_tiles = []

                def emit_attnv(g, qbs=range(NB)):
                    # one accumulation group per av PSUM bank: start only on
                    # the tile's very first write (zeroes the whole 2KB zero
                    # region), stop only on its very last
                    for qb in qbs:
                        for h01 in range(2):
                            nc.tensor.matmul(
                                av[qb // 2][:, qb % 2,
                                            (HD + 1) * h01:(HD + 1) * (h01 + 1)],
                                exp_tiles[g][:, h01, P * qb:P * (qb + 1)],
                                vh_tiles[g][:, 2 * pair + h01, :],
                                start=(g == 0 and qb % 2 == 0 and h01 == 0),
                                stop=(g == n_kv - 1 and qb % 2 == 1 and h01 == 1))

                for g in range(n_kv):
                    sc_t = ps.tile([P, 2, NQ], F32, name="ps_t", tag="ps")
                    for h01 in range(2):
                        nc.tensor.matmul(
                            sc_t[:, h01, :],
                            khT[pair][HD * h01:HD * (h01 + 1), P * g:P * (g + 1)],
                            qhT[pair][HD * h01:HD * (h01 + 1), :],
                            start=True, stop=True,
                            tile_position=(HD * h01, 0))
                    e_t = expp.tile([P, 2, NQ], BF16, name="exp_t", tag="exp")
                    nc.scalar.activation(
                        e_t, sc_t, mybir.ActivationFunctionType.Exp)
                    exp_tiles.append(e_t)
                    if g >= 2:
                        emit_attnv(g - 2)
                    if g >= 3:
                        pop_one()

                rcp_t = rcpp.tile([P, 2, 2, 2], F32, name="rcp_t", tag="rcp")
                nrm_t = nrmp.tile([P, NB, P], BF16, name="nrm_t", tag="nrm")

                def emit_norm(i):
                    nc.vector.reciprocal(rcp_t[:, i, :, :], av[i][:, :, HD:HD + 2])
                    for qb in (2 * i, 2 * i + 1):
                        nc.vector.tensor_scalar_mul(
                            nrm_t[:, qb, 0:HD],
                            av[qb // 2][:, qb % 2, 0:HD],
                            rcp_t[:, qb // 2, qb % 2, 0:1])
                        nc.vector.tensor_scalar_mul(
                            nrm_t[:, qb, HD:2 * HD],
                            av[qb // 2][:, qb % 2, HD + 2:2 * (HD + 1)],
                            rcp_t[:, qb // 2, qb % 2, 1:2])

                emit_attnv(n_kv - 2)
                emit_attnv(n_kv - 1, qbs=(0, 1))
                emit_norm(0)
                emit_attnv(n_kv - 1, qbs=(2, 3))
                emit_norm(1)

                stk_t = stkp.tile([P, NQ], BF16, name="stk_t", tag="stk")
                return nrm_t, stk_t

            # ---------- program emission with cross-repeat pipelining ------
            xk_tiles, go_xk = x_dma_closure(xtk, "xk")
            go_xk()
            nc.sync.dma_start(out=wv_t, in_=wv.ap().rearrange("(t p) m -> p t m", p=P))
            xv_tiles, go_xv = x_dma_closure(xtv, "xv")
            go_xv()
            nc.sync.dma_start(out=wq_t, in_=wq.ap().rearrange("(t p) m -> p t m", p=P))
            xq_tiles, go_xq = x_dma_closure(xtq, "xq")
            go_xq()
            nc.sync.dma_start(out=wo_t, in_=wo.ap().rearrange("(t p) m -> p t m", p=P))
            nc.sync.dma_start(out=id_t, in_=ident.ap())

            khT, kcls = kproj_closures(xk_tiles)
            for c in kcls:
                c()
            vh_tiles, vcls = vproj_closures(xv_tiles)
            for c in vcls:
                c()

            qhT, qcls = qproj_closures(xq_tiles, 0)
            for c in qcls:
                c()

            for rep in range(repeat):
                for qc in range(n_qc):
                    if qc == 1 and rep + 1 < repeat:
                        # stage next repeat's loads + projections as deferred
                        nxk, go_nxk = x_dma_closure(xtk, "xk")
                        nxv, go_nxv = x_dma_closure(xtv, "xv")
                        nxq, go_nxq = x_dma_closure(xtq, "xq")
                        nkhT, nkcls = kproj_closures(nxk)
                        nvh, nvcls = vproj_closures(nxv)
                        deferred.append(go_nxk)
                        deferred.extend(nkcls)
                        deferred.append(go_nxv)
                        deferred.extend(nvcls)
                        deferred.append(go_nxq)
                        next_state = (nkhT, nvh, nxq)
                    stk_tiles = []
                    for pair in range(n_m):
                        if pair == 1:
                            # queue the NEXT chunk's Q projection so it pops
                            # inside this pair's loop (Act keeps streaming)
                            if qc + 1 < n_qc:
                                nqhT, nqcls = qproj_closures(xq_tiles, qc + 1)
                                deferred.extend(nqcls)
                            elif rep + 1 < repeat:
                                nqhT, nqcls = qproj_closures(next_state[2], 0)
                                deferred.extend(nqcls)
                                deferred.append(
                                    lambda: prereqs_done.__setitem__(0, True))
                            else:
                                nqhT = None
                        nrm_t, stk_t = emit_pair_loop(khT, vh_tiles, qhT, pair)
                        stk_tiles.append(stk_t)
                        deferred.append(transpose_closure(nrm_t, stk_t))
                    deferred.extend(outproj_closures(stk_tiles, qc))
                    qhT = nqhT
                # rep boundary: drain only what the next rep's first loop
                # depends on (khT/vh/x DMAs/qproj are FIFO-ahead of the
                # barrier); the remaining tail work pops inside the next
                # rep's chunk-0 loops, keeping the PE fed there.
                if rep + 1 < repeat:
                    while not prereqs_done[0]:
                        pop_one()
                    prereqs_done[0] = False
                    khT, vh_tiles, xq_tiles = next_state
                else:
                    flush()
    nc.finalize()
    return nc


_PROG_CACHE = {}


def _get_program(seq=SEQ, repeat=1):
    key = (seq, repeat)
    if key not in _PROG_CACHE:
        _PROG_CACHE[key] = build_program(seq, repeat)
    return _PROG_CACHE[key]


def shard_inputs(q, k, v, Wq, Wk, Wv, Wo, seq=SEQ):
    """Build the 8 per-core input maps (host-side layout prep, bf16)."""
    bf = ml_dtypes.bfloat16
    scale = np.float32(QK_SCALE / math.sqrt(B))
    xt = {}
    for b in range(B):
        xt[b] = (np.ascontiguousarray(q[b][:seq].T).astype(bf),
                 np.ascontiguousarray(k[b][:seq].T).astype(bf),
                 np.ascontiguousarray(v[b][:seq].T).astype(bf))
    id_np = np.eye(P, dtype=np.float32).astype(bf)
    in_maps = []
    for c in range(N_CORES):
        b = c // 4
        hg = c % 4
        heads = [4 * hg + j for j in range(HPC)]
        wq_s = np.concatenate([Wq[:, h::NUM_HEADS] for h in heads], axis=1) * scale
        wk_s = np.concatenate([Wk[:, h::NUM_HEADS] for h in heads], axis=1)
        wv_s = np.zeros((E, HPC, HD + 1), dtype=np.float32)
        for j, h in enumerate(heads):
            if j % 2 == 0:
                wv_s[:, j, 0:HD] = Wv[:, h::NUM_HEADS]
            else:
                wv_s[:, j, 1:HD + 1] = Wv[:, h::NUM_HEADS]
        wo_s = np.concatenate([Wo[h::NUM_HEADS, :] for h in heads], axis=0)
        in_maps.append({
            "xtq": xt[b][0],
            "xtk": xt[b][1],
            "xtv": xt[b][2],
            "wq": np.ascontiguousarray(wq_s).astype(bf),
            "wk": np.ascontiguousarray(wk_s).astype(bf),
            "wv": np.ascontiguousarray(wv_s.reshape(E, HPC * (HD + 1))).astype(bf),
            "wo": np.ascontiguousarray(wo_s).astype(bf),
            "ident": id_np,
        })
    return in_maps


def unshard(results, seq=SEQ):
    out = np.zeros((B, seq, E), dtype=np.float32)
    for c in range(N_CORES):
        out[c // 4] += results[c]["out"]
    return out


def kernel(q, k, v, Wq, Wk, Wv, Wo):
    q = np.asarray(q, dtype=np.float32)
    k = np.asarray(k, dtype=np.float32)
    v = np.asarray(v, dtype=np.float32)
    Wq = np.asarray(Wq, dtype=np.float32)
    Wk = np.asarray(Wk, dtype=np.float32)
    Wv = np.asarray(Wv, dtype=np.float32)
    Wo = np.asarray(Wo, dtype=np.float32)
    nc = _get_program()
    in_maps = shard_inputs(q, k, v, Wq, Wk, Wv, Wo)
    res = run_bass_kernel_spmd(nc, in_maps, list(range(N_CORES)))
    return unshard(res.results)


# revision 5
# speedup vs baseline: 1.3265x; 1.1489x over previous
"""Trainium2 Bass kernel for nn_AttnDecoder_87230785782556 — v3.

Multi-head attention decoder: out = softmax((xq Wq)(xk Wk)^T * s) (xv Wv) Wo
Sharding: 8 cores = 2 batches x 4 head-groups (tensor-parallel heads,
row-split Wo; partial outputs summed on host).

Pipeline design (per core):
- bf16 on-chip (PSUM f32): halves DMA + SBUF vs f32.
- scores [kv, q] in PSUM -> exp on Act engine (the rate limiter: ~33us
  per chunk) -> attnV with exp STATIONARY and vh moving (65 rows per
  matmul: half the PE rows of the classic orientation). The vh ones
  column emits softmax denominators into the same PSUM accumulator.
- normalize = per-partition scalar multiply on the Pool engine during
  PSUM evacuation; PE transpose returns attn to [feat, q] for the
  output projection.
- ALL non-loop PE work (K/V/Q projections, transposes, output
  projection) is sliced into small closures and interleaved into the
  Act-bound kv loops, including the NEXT repeat's projections, so the
  PE never idles at phase or repeat boundaries.
- output DMA rides the Pool DGE queue so a blocked x-load on the SP
  queue can never delay result writeback.
"""
import math
import numpy as np
import ml_dtypes

from concourse import bacc, mybir, tile
from concourse.bass_utils import run_bass_kernel_spmd

B = 2
SEQ = 2048
E = 1024
NUM_HEADS = 16
HD = 64
QK_SCALE = 0.125
N_CORES = 8
HPC = 4            # heads per core
P = 128
NQ = 512           # q chunk

F32 = mybir.dt.float32
BF16 = mybir.dt.bfloat16


def build_program(seq=SEQ, repeat=1):
    nc = bacc.Bacc("TRN2", target_bir_lowering=False, debug=False,
                   num_devices=N_CORES)

    n_qc = seq // NQ            # q chunks (4)
    n_kv = seq // P             # kv tiles of 128 (16)
    n_kt = E // P               # contraction tiles (8)
    FPC = HPC * HD              # features per core (256)
    n_m = FPC // P              # feature pair-tiles (2)
    NB = NQ // P                # q 128-blocks per chunk (4)
    VW = HPC * (HD + 1)         # vh row width (260)

    xtq = nc.dram_tensor("xtq", [E, seq], BF16, kind="ExternalInput")
    xtk = nc.dram_tensor("xtk", [E, seq], BF16, kind="ExternalInput")
    xtv = nc.dram_tensor("xtv", [E, seq], BF16, kind="ExternalInput")
    wq = nc.dram_tensor("wq", [E, FPC], BF16, kind="ExternalInput")
    wk = nc.dram_tensor("wk", [E, FPC], BF16, kind="ExternalInput")
    wv = nc.dram_tensor("wv", [E, VW], BF16, kind="ExternalInput")
    wo = nc.dram_tensor("wo", [FPC, E], BF16, kind="ExternalInput")
    ident = nc.dram_tensor("ident", [P, P], BF16, kind="ExternalInput")
    out = nc.dram_tensor("out", [seq, E], F32, kind="ExternalOutput")

    with tile.TileContext(nc) as tc, nc.allow_low_precision("bf16 pipeline"):
        import contextlib
        ctx = contextlib.ExitStack()
        with ctx:
            consts = ctx.enter_context(tc.tile_pool(name="consts", bufs=1))
            bigs = ctx.enter_context(tc.tile_pool(name="bigs", bufs=2))
            vhp = ctx.enter_context(tc.tile_pool(name="vhp", bufs=2 * n_kv))
            qhp = ctx.enter_context(tc.tile_pool(name="qhp", bufs=4))
            xs = ctx.enter_context(tc.tile_pool(name="xs", bufs=1))
            expp = ctx.enter_context(tc.tile_pool(name="expp", bufs=8))
            nrmp = ctx.enter_context(tc.tile_pool(name="nrmp", bufs=4))
            stkp = ctx.enter_context(tc.tile_pool(name="stkp", bufs=4))
            rcpp = ctx.enter_context(tc.tile_pool(name="rcpp", bufs=4))
            finp = ctx.enter_context(tc.tile_pool(name="finp", bufs=3))
            ps = ctx.enter_context(tc.tile_pool(name="ps", bufs=2, space="PSUM"))
            avp = ctx.enter_context(tc.tile_pool(name="avp", bufs=2, space="PSUM"))
            opp = ctx.enter_context(tc.tile_pool(name="opp", bufs=2, space="PSUM"))

            # ---- resident constants (wk first: K proj is the first consumer)
            wq_t = consts.tile([P, n_kt, FPC], BF16, name="wq_t", tag="wq")
            wk_t = consts.tile([P, n_kt, FPC], BF16, name="wk_t", tag="wk")
            wv_t = consts.tile([P, n_kt, VW], BF16, name="wv_t", tag="wv")
            wo_t = consts.tile([P, n_m, E], BF16, name="wo_t", tag="wo")
            id_t = consts.tile([P, P], BF16, name="id_t", tag="id")
            nc.sync.dma_start(out=wk_t, in_=wk.ap().rearrange("(t p) m -> p t m", p=P))

            deferred = []
            prereqs_done = [False]

            def pop_one():
                if deferred:
                    deferred.pop(0)()

            def flush():
                while deferred:
                    deferred.pop(0)()

            # ---------- closure factories (allocate tiles eagerly, emit
            # instructions when called) ----------
            def x_dma_closure(dram, tag):
                tiles = [xs.tile([P, seq], BF16, name=f"{tag}{kt}",
                                 tag=f"{tag}{kt}") for kt in range(n_kt)]

                def go():
                    for kt in range(n_kt):
                        nc.sync.dma_start(out=tiles[kt],
                                          in_=dram.ap()[P * kt:P * (kt + 1), :])
                return tiles, go

            def kproj_closures(xk_tiles):
                """khT[m] [128, seq] bf16; one closure per (nq, m)."""
                khT = [bigs.tile([P, seq], BF16, name=f"khT{m}", tag=f"khT{m}")
                       for m in range(n_m)]
                cls = []
                for nq in range(n_qc):
                    for m in range(n_m):
                        box = {}

                        def go_a(nq=nq, m=m, box=box):
                            box["pt"] = opp.tile([P, NQ], F32, name="op_t", tag="op")
                            for kt in range(n_kt // 2):
                                nc.tensor.matmul(
                                    box["pt"],
                                    wk_t[:, kt, P * m:P * (m + 1)],
                                    xk_tiles[kt][:, NQ * nq:NQ * (nq + 1)],
                                    start=(kt == 0), stop=False)

                        def go_b(nq=nq, m=m, box=box):
                            for kt in range(n_kt // 2, n_kt):
                                nc.tensor.matmul(
                                    box["pt"],
                                    wk_t[:, kt, P * m:P * (m + 1)],
                                    xk_tiles[kt][:, NQ * nq:NQ * (nq + 1)],
                                    start=False, stop=(kt == n_kt - 1))
                            nc.vector.tensor_copy(
                                khT[m][:, NQ * nq:NQ * (nq + 1)], box["pt"])
                        cls.append(go_a)
                        cls.append(go_b)
                return khT, cls

            def vproj_closures(xv_tiles):
                """vh tiles [128, 4, 65] bf16; one closure per kv tile."""
                vh_tiles = [vhp.tile([P, HPC, HD + 1], BF16, name=f"vh{i}",
                                     tag="vh") for i in range(n_kv)]
                cls = []
                for mk in range(n_kv):
                    def go(mk=mk):
                        pt = opp.tile([P, VW], F32, name="op_t", tag="op")
                        c0 = P * mk
                        for kt in range(n_kt):
                            nc.tensor.matmul(
                                pt,
                                xv_tiles[kt][:, c0:c0 + P],
                                wv_t[:, kt, :],
                                start=(kt == 0), stop=(kt == n_kt - 1))
                        nc.vector.tensor_copy(
                            vh_tiles[mk],
                            pt.rearrange("p (h c) -> p h c", h=HPC))
                        nc.gpsimd.tensor_scalar_add(
                            vh_tiles[mk][:, 0::2, HD],
                            vh_tiles[mk][:, 0::2, HD], 1.0)
                        nc.gpsimd.tensor_scalar_add(
                            vh_tiles[mk][:, 1::2, 0],
                            vh_tiles[mk][:, 1::2, 0], 1.0)
                    cls.append(go)
                return vh_tiles, cls

            def qproj_closures(xq_tiles, qc):
                qhT = [qhp.tile([P, NQ], BF16, name=f"qhT{m}", tag=f"qhT{m}")
                       for m in range(n_m)]
                cls = []
                for m in range(n_m):
                    box = {}

                    def go_a(m=m, box=box):
                        box["pt"] = opp.tile([P, NQ], F32, name="op_t", tag="op")
                        for kt in range(n_kt // 2):
                            nc.tensor.matmul(
                                box["pt"],
                                wq_t[:, kt, P * m:P * (m + 1)],
                                xq_tiles[kt][:, NQ * qc:NQ * (qc + 1)],
                                start=(kt == 0), stop=False)

                    def go_b(m=m, box=box):
                        for kt in range(n_kt // 2, n_kt):
                            nc.tensor.matmul(
                                box["pt"],
                                wq_t[:, kt, P * m:P * (m + 1)],
                                xq_tiles[kt][:, NQ * qc:NQ * (qc + 1)],
                                start=False, stop=(kt == n_kt - 1))
                        nc.vector.tensor_copy(qhT[m], box["pt"])
                    cls.append(go_a)
                    cls.append(go_b)
                return qhT, cls

            def transpose_closure(nrm_t, stk_t):
                def go():
                    tp_t = opp.tile([P, NB, P], BF16, name="tp_t", tag="op")
                    for qb in range(NB):
                        # all 4 transposes share one PSUM bank: single
                        # accumulation group (start zeroes the zero region)
                        nc.tensor.matmul(
                            tp_t[:, qb, :], nrm_t[:, qb, :], id_t,
                            is_transpose=True,
                            start=(qb == 0), stop=(qb == NB - 1))
                    nc.vector.tensor_copy(
                        stk_t, tp_t.rearrange("p a b -> p (a b)"))
                return go

            def outproj_closures(stk_tiles, qc):
                cls = []
                for qs in range(NB):
                    box = {}

                    def go_a(qs=qs, box=box):
                        box["fin"] = finp.tile([P, 2, NQ], F32, name="fin_t",
                                               tag="fin")
                        op_ps = opp.tile([P, NQ], F32, name="op_t", tag="op")
                        for pair in range(n_m):
                            nc.tensor.matmul(
                                op_ps,
                                stk_tiles[pair][:, P * qs:P * (qs + 1)],
                                wo_t[:, pair, 0:NQ],
                                start=(pair == 0), stop=(pair == n_m - 1))
                        nc.vector.tensor_copy(box["fin"][:, 0, :], op_ps)

                    def go_b(qs=qs, box=box):
                        op_ps = opp.tile([P, NQ], F32, name="op_t", tag="op")
                        for pair in range(n_m):
                            nc.tensor.matmul(
                                op_ps,
                                stk_tiles[pair][:, P * qs:P * (qs + 1)],
                                wo_t[:, pair, NQ:2 * NQ],
                                start=(pair == 0), stop=(pair == n_m - 1))
                        nc.vector.tensor_copy(box["fin"][:, 1, :], op_ps)
                        r0 = NQ * qc + P * qs
                        nc.gpsimd.dma_start(
                            out=out.ap()[r0:r0 + P, :],
                            in_=box["fin"].rearrange("p a b -> p (a b)"))
                    cls.append(go_a)
                    cls.append(go_b)
                return cls

            def emit_pair_loop(khT, vh_tiles, qhT, pair):
                """scores -> exp -> attnV (lag 2) for one head pair; returns
                (nrm_t, stk_t) with normalize already emitted."""
                av = [avp.tile([P, 2, 2 * (HD + 1)], F32,
                               name=f"av{i}", tag="av") for i in range(2)]
                exp_tiles = []

                def emit_attnv(g, qbs=range(NB)):
                    # one accumulation group per av PSUM bank: start only on
                    # the tile's very first write (zeroes the whole 2KB zero
                    # region), stop only on its very last
                    for qb in qbs:
                        for h01 in range(2):
                            nc.tensor.matmul(
                                av[qb // 2][:, qb % 2,
                                            (HD + 1) * h01:(HD + 1) * (h01 + 1)],
                                exp_tiles[g][:, h01, P * qb:P * (qb + 1)],
                                vh_tiles[g][:, 2 * pair + h01, :],
                                start=(g == 0 and qb % 2 == 0 and h01 == 0),
                                stop=(g == n_kv - 1 and qb % 2 == 1 and h01 == 1))

                for g in range(n_kv):
                    sc_t = ps.tile([P, 2, NQ], F32, name="ps_t", tag="ps")
                    for h01 in range(2):
                        nc.tensor.matmul(
                            sc_t[:, h01, :],
                            khT[pair][HD * h01:HD * (h01 + 1), P * g:P * (g + 1)],
                            qhT[pair][HD * h01:HD * (h01 + 1), :],
                            start=True, stop=True,
                            tile_position=(HD * h01, 0))
                    e_t = expp.tile([P, 2, NQ], BF16, name="exp_t", tag="exp")
                    nc.scalar.activation(
                        e_t, sc_t, mybir.ActivationFunctionType.Exp)
                    exp_tiles.append(e_t)
                    if g >= 3:
                        emit_attnv(g - 3)
                    if g >= 3:
                        pop_one()

                rcp_t = rcpp.tile([P, 2, 2, 2], F32, name="rcp_t", tag="rcp")
                nrm_t = nrmp.tile([P, NB, P], BF16, name="nrm_t", tag="nrm")

                def emit_norm(i):
                    nc.vector.reciprocal(rcp_t[:, i, :, :], av[i][:, :, HD:HD + 2])
                    for qb in (2 * i, 2 * i + 1):
                        nc.vector.tensor_scalar_mul(
                            nrm_t[:, qb, 0:HD],
                            av[qb // 2][:, qb % 2, 0:HD],
                            rcp_t[:, qb // 2, qb % 2, 0:1])
                        nc.vector.tensor_scalar_mul(
                            nrm_t[:, qb, HD:2 * HD],
                            av[qb // 2][:, qb % 2, HD + 2:2 * (HD + 1)],
                            rcp_t[:, qb // 2, qb % 2, 1:2])

                emit_attnv(n_kv - 3)
                emit_attnv(n_kv - 2)
                emit_attnv(n_kv - 1, qbs=(0, 1))
                emit_norm(0)
                emit_attnv(n_kv - 1, qbs=(2, 3))
                emit_norm(1)

                stk_t = stkp.tile([P, NQ], BF16, name="stk_t", tag="stk")
                return nrm_t, stk_t

            # ---------- program emission with cross-repeat pipelining ------
            xk_tiles, go_xk = x_dma_closure(xtk, "xk")
            go_xk()
            nc.sync.dma_start(out=wv_t, in_=wv.ap().rearrange("(t p) m -> p t m", p=P))
            xv_tiles, go_xv = x_dma_closure(xtv, "xv")
            go_xv()
            nc.sync.dma_start(out=wq_t, in_=wq.ap().rearrange("(t p) m -> p t m", p=P))
            xq_tiles, go_xq = x_dma_closure(xtq, "xq")
            go_xq()
            nc.sync.dma_start(out=wo_t, in_=wo.ap().rearrange("(t p) m -> p t m", p=P))
            nc.sync.dma_start(out=id_t, in_=ident.ap())

            khT, kcls = kproj_closures(xk_tiles)
            for c in kcls:
                c()
            vh_tiles, vcls = vproj_closures(xv_tiles)
            for c in vcls:
                c()

            qhT, qcls = qproj_closures(xq_tiles, 0)
            for c in qcls:
                c()

            for rep in range(repeat):
                for qc in range(n_qc):
                    if qc == 1 and rep + 1 < repeat:
                        # stage next repeat's loads + projections as deferred
                        nxk, go_nxk = x_dma_closure(xtk, "xk")
                        nxv, go_nxv = x_dma_closure(xtv, "xv")
                        nxq, go_nxq = x_dma_closure(xtq, "xq")
                        nkhT, nkcls = kproj_closures(nxk)
                        nvh, nvcls = vproj_closures(nxv)
                        deferred.append(go_nxk)
                        deferred.extend(nkcls)
                        deferred.append(go_nxv)
                        deferred.extend(nvcls)
                        deferred.append(go_nxq)
                        next_state = (nkhT, nvh, nxq)
                    stk_tiles = []
                    for pair in range(n_m):
                        if pair == 1:
                            # queue the NEXT chunk's Q projection so it pops
                            # inside this pair's loop (Act keeps streaming)
                            if qc + 1 < n_qc:
                                nqhT, nqcls = qproj_closures(xq_tiles, qc + 1)
                                deferred.extend(nqcls)
                            elif rep + 1 < repeat:
                                nqhT, nqcls = qproj_closures(next_state[2], 0)
                                deferred.extend(nqcls)
                                deferred.append(
                                    lambda: prereqs_done.__setitem__(0, True))
                            else:
                                nqhT = None
                        nrm_t, stk_t = emit_pair_loop(khT, vh_tiles, qhT, pair)
                        stk_tiles.append(stk_t)
                        deferred.append(transpose_closure(nrm_t, stk_t))
                    deferred.extend(outproj_closures(stk_tiles, qc))
                    qhT = nqhT
                # rep boundary: drain only what the next rep's first loop
                # depends on (khT/vh/x DMAs/qproj are FIFO-ahead of the
                # barrier); the remaining tail work pops inside the next
                # rep's chunk-0 loops, keeping the PE fed there.
                if rep + 1 < repeat:
                    while not prereqs_done[0]:
                        pop_one()
                    prereqs_done[0] = False
                    khT, vh_tiles, xq_tiles = next_state
                else:
                    flush()
    nc.finalize()
    return nc


_PROG_CACHE = {}


def _get_program(seq=SEQ, repeat=1):
    key = (seq, repeat)
    if key not in _PROG_CACHE:
        _PROG_CACHE[key] = build_program(seq, repeat)
    return _PROG_CACHE[key]


def shard_inputs(q, k, v, Wq, Wk, Wv, Wo, seq=SEQ):
    """Build the 8 per-core input maps (host-side layout prep, bf16)."""
    bf = ml_dtypes.bfloat16
    scale = np.float32(QK_SCALE / math.sqrt(B))
    xt = {}
    for b in range(B):
        xt[b] = (np.ascontiguousarray(q[b][:seq].T).astype(bf),
                 np.ascontiguousarray(k[b][:seq].T).astype(bf),
                 np.ascontiguousarray(v[b][:seq].T).astype(bf))
    id_np = np.eye(P, dtype=np.float32).astype(bf)
    in_maps = []
    for c in range(N_CORES):
        b = c // 4
        hg = c % 4
        heads = [4 * hg + j for j in range(HPC)]
        wq_s = np.concatenate([Wq[:, h::NUM_HEADS] for h in heads], axis=1) * scale
        wk_s = np.concatenate([Wk[:, h::NUM_HEADS] for h in heads], axis=1)
        wv_s = np.zeros((E, HPC, HD + 1), dtype=np.float32)
        for j, h in enumerate(heads):
            if j % 2 == 0:
                wv_s[:, j, 0:HD] = Wv[:, h::NUM_HEADS]
            else:
                wv_s[:, j, 1:HD + 1] = Wv[:, h::NUM_HEADS]
        wo_s = np.concatenate([Wo[h::NUM_HEADS, :] for h in heads], axis=0)
        in_maps.append({
            "xtq": xt[b][0],
            "xtk": xt[b][1],
            "xtv": xt[b][2],
            "wq": np.ascontiguousarray(wq_s).astype(bf),
            "wk": np.ascontiguousarray(wk_s).astype(bf),
            "wv": np.ascontiguousarray(wv_s.reshape(E, HPC * (HD + 1))).astype(bf),
            "wo": np.ascontiguousarray(wo_s).astype(bf),
            "ident": id_np,
        })
    return in_maps


def unshard(results, seq=SEQ):
    out = np.zeros((B, seq, E), dtype=np.float32)
    for c in range(N_CORES):
        out[c // 4] += results[c]["out"]
    return out


def kernel(q, k, v, Wq, Wk, Wv, Wo):
    q = np.asarray(q, dtype=np.float32)
    k = np.asarray(k, dtype=np.float32)
    v = np.asarray(v, dtype=np.float32)
    Wq = np.asarray(Wq, dtype=np.float32)
    Wk = np.asarray(Wk, dtype=np.float32)
    Wv = np.asarray(Wv, dtype=np.float32)
    Wo = np.asarray(Wo, dtype=np.float32)
    nc = _get_program()
    in_maps = shard_inputs(q, k, v, Wq, Wk, Wv, Wo)
    res = run_bass_kernel_spmd(nc, in_maps, list(range(N_CORES)))
    return unshard(res.results)
